# revision 25
# baseline (speedup 1.0000x reference)
"""Multi-head attention (B=2, S=2048, D=1024, H=16) on 8 NeuronCores.

Sharding: core c -> (batch b = c//4, head-group hg = c%4 of 4 heads).
Each core: QKV projection for its 4 heads (bf16, f32 PSUM), transposed-score
flash attention (softmax denominator via an appended ones-column on V), and
the out-projection restricted to its heads' rows of out_w.  Host sums the 4
per-head-group fp16 partials per batch and adds out_b (exact, linear).

v3 schedule: the kernel is a single interleaved era paced by the ScalarE
exp stream (the true floor at ~80us/core).  Attention is split into a
scores+exp generator and an AV generator per (pair, q-half); large PE
blocks (late projection chunks, V-projection, out-projection) are emitted
between score chunks -- never inside an open PSUM accumulation group, and
out-projection only while no ctx group is open (mixing matmul groups into
open groups corrupts PSUM on HW -- measured).  Each half's AV phase rides
inside the next half's score phase, where its own scores+exp singles are
the (proven-safe) foreign work.

Other v3/v2 mechanics: host-retiled inputs -> one large DMA per tensor;
score matmuls head-interleaved at row-groups 0/64 (concurrent halves of
the PE array, keeps the HAM clock-gate open); exp merged over both heads
per 512-col chunk; fp16 partial outputs (host accumulates in f32).
"""

import os
import sys

sys.path.insert(0, "/opt/trn_rl_repo")

import numpy as np
import ml_dtypes

import concourse.bass as bass  # noqa: F401  (AP helpers)
import concourse.mybir as mybir
import concourse.tile as tile
from concourse import bacc
from concourse.bass_utils import run_bass_kernel_spmd
from concourse.masks import make_upper_triangular

B, S, D, H, DH = 2, 2048, 1024, 16, 64
NCORES = 8
HPC = 4            # heads per core
EQ = HPC * DH      # 256: q (or k, or v) columns per core
E = 3 * EQ         # 768: total projected columns per core
HQ = S // 2        # 1024: columns per q-half
BF16 = mybir.dt.bfloat16
F16 = mybir.dt.float16
F32 = mybir.dt.float32
NP_BF16 = ml_dtypes.bfloat16
EXPFN = mybir.ActivationFunctionType.Exp

_prog_cache: dict = {}
last_results = None  # BassKernelResults of the most recent run (for test.py)


def _emit(tc, xt_h, w_h, wo_h, out_h, causal, nd):
    nc = tc.nc

    with (
        tc.tile_pool(name="persist", bufs=1) as pp,
        tc.tile_pool(name="pt", bufs=20) as pt_pool,
        tc.tile_pool(name="norm", bufs=1) as norm_pool,
        tc.tile_pool(name="outsb", bufs=3) as out_pool,
        tc.tile_pool(name="dram", bufs=2, space="DRAM") as dram_pool,
        tc.tile_pool(name="ctxp", bufs=1, space="PSUM") as ctxp,
        tc.tile_pool(name="stp", bufs=2, space="PSUM") as stp,
    ):
        # ---- persistent SBUF tensors ----
        xt_sb = pp.tile([128, 4, nd, 512], BF16, tag="xt", name="xt_sb")
        w_sb = pp.tile([128, nd, E], BF16, tag="w", name="w_sb")
        wo_sb = pp.tile([128, 2, D], BF16, tag="wo", name="wo_sb")
        qkT_sb = pp.tile([128, 4, S], BF16, tag="qkT", name="qkT_sb")
        # V' per (k-tile j, head h): [128, 65], col 64 = ones (softmax denom)
        vp_sb = pp.tile([128, 16, HPC, 65], BF16, tag="vp", name="vp_sb")
        ctx_all = pp.tile([128, 2, S], BF16, tag="ctx", name="ctx_all")

        # one large DMA per tensor: 128 long contiguous partition lines
        nc.sync.dma_start(out=w_sb, in_=w_h[:, :, :])
        for ch in range(4):
            nc.sync.dma_start(out=xt_sb[:, ch], in_=xt_h[:, ch, :, :])
        nc.sync.dma_start(out=wo_sb, in_=wo_h[:, :, :])

        if causal:
            tri_f = pp.tile([128, 128], F32, tag="trif", name="tri_f")
            make_upper_triangular(nc, tri_f, val=1.0, diag=True)
            tri_bf = pp.tile([128, 128], BF16, tag="trib", name="tri_bf")
            nc.vector.tensor_copy(tri_bf, tri_f)

        nc.vector.memset(vp_sb[:, :, :, 64:65], 1.0)

        # Pre-warm the PE clock gate during the input-DMA window.
        warm_sb = pp.tile([128, 128], BF16, tag="warm", name="warm_sb")
        nc.vector.memset(warm_sb, 0.0)
        warm_ps = stp.tile([128, 2, 512], F32, tag="st", name="warm_ps")
        for _ in range(24):
            nc.tensor.matmul(
                warm_ps[:, 0, 0:128], lhsT=warm_sb, rhs=warm_sb,
                start=True, stop=True,
            )

        # ---- fill-work generators (complete matmul groups per yield) ----

        # The projection fill units borrow the ctx0/ctx1 PSUM allocations
        # (the only way to stay within 8 banks: st 2x2 + ctx 2x2). They are
        # fully drained before any AV phase opens ctx accumulation groups.
        _filln = [0]

        def _fil_tile(name):
            t = _filln[0] % 2
            _filln[0] += 1
            return ctxp.tile([128, HQ], F32, tag=f"ctx{t}", name=name)

        def proj_qk_chunk(ch, ets=(0, 1, 2, 3)):
            """Q^T/K^T projection for one 512-col s-chunk, one e-tile/yield."""
            for et in ets:
                fil = _fil_tile("qk_ps")
                ps = fil[:, 0:512]
                for d in range(nd):
                    nc.tensor.matmul(
                        ps,
                        lhsT=w_sb[:, d, 128 * et : 128 * (et + 1)],
                        rhs=xt_sb[:, ch, d, :],
                        start=(d == 0),
                        stop=(d == nd - 1),
                    )
                nc.vector.tensor_copy(
                    qkT_sb[:, et, 512 * ch : 512 * (ch + 1)], ps
                )
                yield

        def proj_v(lo, hi):
            """V in [s, e_v] layout for all 4 heads; one s-tile per yield."""
            for i in range(lo, hi):
                ch, io = i // 4, i % 4
                fil = _fil_tile("v_ps")
                vps = fil[:, 0:256]
                for d in range(nd):
                    nc.tensor.matmul(
                        vps,
                        lhsT=xt_sb[:, ch, d, 128 * io : 128 * (io + 1)],
                        rhs=w_sb[:, d, 2 * EQ : 3 * EQ],
                        start=(d == 0),
                        stop=(d == nd - 1),
                    )
                nc.vector.tensor_copy(
                    vp_sb[:, i, :, 0:64],
                    vps.rearrange("p (h e) -> p h e", h=HPC),
                )
                yield

        def outproj(lo, hi, split_cast=False):
            """Out-projection + fp16 store for s-tiles [lo, hi).

            Must only run while NO ctx accumulation group is open.
            """
            for i in range(lo, hi):
                ops = stp.tile([128, 2, 512], F32, tag="st", name="o_ps")
                for et in range(2):
                    for c in range(2):
                        nc.tensor.matmul(
                            ops[:, c, :],
                            lhsT=ctx_all[:, et, 128 * i : 128 * (i + 1)],
                            rhs=wo_sb[:, et, 512 * c : 512 * (c + 1)],
                            start=(et == 0),
                            stop=(et == 1),
                        )
                osb = out_pool.tile([128, 2, 512], F16, tag="osb", name="o_sb")
                if split_cast:
                    nc.vector.tensor_copy(osb[:, 0, :], ops[:, 0, :])
                    nc.scalar.copy(osb[:, 1, :], ops[:, 1, :])
                else:
                    nc.vector.tensor_copy(osb, ops)
                nc.sync.dma_start(
                    out=out_h[128 * i : 128 * (i + 1), :],
                    in_=osb.rearrange("p a b -> p (a b)"),
                )
                yield

        # ---- attention, split into scores+exp and AV generators ----

        def jrange(half):
            Q0, Q1 = HQ * half, HQ * (half + 1)
            return [j for j in range(16) if not (causal and 128 * j >= Q1)], Q0, Q1

        def scores(p, half, pts_out):
            """Scores+exp for all j of (pair, half); yields per 512-chunk.

            Emits only single (start=stop=True) matmuls + ACT/gpsimd work,
            so it is safe to interleave with anything.
            Appends (j, pts) to pts_out as each j's exp completes.
            """
            qsl = [qkT_sb[hh * 64 : hh * 64 + 64, p, :] for hh in range(2)]
            ksl = [qkT_sb[hh * 64 : hh * 64 + 64, 2 + p, :] for hh in range(2)]
            js, Q0, Q1 = jrange(half)
            for j in js:
                q0 = max(128 * j, Q0) if causal else Q0
                qcols = Q1 - q0
                pts = pt_pool.tile([128, 2, HQ], BF16, tag="pt", name="pt")
                off = 0
                while off < qcols:
                    clen = min(512, qcols - off)
                    st = stp.tile([128, 2, 512], F32, tag="st", name="st_ps")
                    for hh in range(2):
                        nc.tensor.matmul(
                            st[:, hh, 0:clen],
                            lhsT=ksl[hh][:, 128 * j : 128 * (j + 1)],
                            rhs=qsl[hh][:, q0 + off : q0 + off + clen],
                            start=True,
                            stop=True,
                        )
                    nc.scalar.activation(
                        pts[:, :, off : off + clen],
                        st[:, :, 0:clen],
                        EXPFN,
                        scale=0.125,
                    )
                    off += clen
                    if off < qcols:
                        yield (352 + 2 * clen) / 1.2
                if causal and 128 * j >= Q0:
                    for hh in range(2):
                        nc.gpsimd.tensor_mul(
                            pts[:, hh, 0:128], pts[:, hh, 0:128], tri_bf
                        )
                pts_out.append((j, pts))
                yield (352 + 2 * clen) / 1.2

        def _norm_chunk(p, hh, ctx_ps, Q0, c):
            """Evacuate + normalize one 512-col ctx chunk of one head."""
            po = hh * 64
            cs = slice(512 * c, 512 * (c + 1))
            ctxu = norm_pool.tile([65, 512], F32, tag=f"ecu{c}{hh}", name="ecu")
            nc.vector.tensor_copy(ctxu, ctx_ps[hh][0:65, cs])
            den_sp = norm_pool.tile([128, 4], F32, tag=f"eds{c}{hh}", name="eds")
            nc.sync.dma_start(out=den_sp, in_=ctxu[64:65, :])
            rec_sp = norm_pool.tile([128, 4], F32, tag=f"ers{c}{hh}", name="ers")
            nc.vector.reciprocal(rec_sp, den_sp)
            rec_d = dram_pool.tile([512], F32, tag=f"erd{c}{hh}", name="erd")
            nc.sync.dma_start(out=rec_d, in_=rec_sp)
            recb = norm_pool.tile([64, 512], F32, tag=f"erb{c}{hh}", name="erb")
            rec_bcast = bass.AP(
                tensor=rec_d.tensor, offset=rec_d.offset,
                ap=[[0, 64]] + list(rec_d.ap),
            )
            nc.sync.dma_start(out=recb, in_=rec_bcast)
            nc.vector.tensor_mul(
                ctx_all[po : po + 64, p, Q0 + 512 * c : Q0 + 512 * (c + 1)],
                ctxu[0:64, :], recb,
            )

        def av(p, half, pts_in, early=False):
            """AV + evac + normalize for (pair, half); consumes pts_in.

            Opens ctx accumulation groups: between this generator's first
            and last AV yield, only `scores` work may interleave.
            """
            js, Q0, Q1 = jrange(half)
            ctx_ps = [
                ctxp.tile([128, HQ], F32, tag=f"ctx{hh}", name=f"ctx_ps{hh}")[
                    0:65, :
                ]
                for hh in range(2)
            ]
            for idx in range(len(js)):
                if idx >= len(pts_in):
                    raise RuntimeError(f"av({p},{half}) starved: idx={idx} len={len(pts_in)}")
                j, pts = pts_in[idx]
                q0 = max(128 * j, Q0) if causal else Q0
                c0 = (q0 - Q0) // 512
                zlen = (q0 - Q0) - 512 * c0
                for hh in range(2):
                    for c in range(c0, 2):
                        gc = 2 * half + c
                        jl = min(15, 4 * gc + 3) if causal else 15
                        if c == c0:
                            dst = ctx_ps[hh][:, 512 * c0 + zlen : 512 * (c0 + 1)]
                            src = pts[:, hh, 0 : 512 - zlen]
                        else:
                            dst = ctx_ps[hh][:, 512 * c : 512 * (c + 1)]
                            src = pts[
                                :, hh,
                                512 * (c - c0) - zlen : 512 * (c - c0 + 1) - zlen,
                            ]
                        nc.tensor.matmul(
                            dst,
                            lhsT=vp_sb[:, j, 2 * p + hh, :],
                            rhs=src,
                            start=(j == 0),
                            stop=(j == jl),
                        )
                if early and causal and idx == 11:
                    # chunk 0 of this half closed at j=11: evacuate and
                    # start its reciprocal chain now, riding the last AVs
                    for hh in range(2):
                        _norm_chunk(p, hh, ctx_ps, Q0, 0)
                yield
            if early and causal:
                for hh in range(2):
                    _norm_chunk(p, hh, ctx_ps, Q0, 1)
                    yield
                return
            # Evacuate unnormalized ctx^T (+ denominator row 64) to SBUF,
            # normalize off-path via a DRAM-bounce reciprocal broadcast.
            for hh in range(2):
                po = hh * 64
                ctxu = norm_pool.tile(
                    [65, HQ], F32, tag=f"ctxu{hh}", name=f"ctxu{hh}"
                )
                nc.vector.tensor_copy(ctxu, ctx_ps[hh])
                den_sp = norm_pool.tile(
                    [128, HQ // 128], F32, tag="densp", name="den_sp"
                )
                nc.sync.dma_start(out=den_sp, in_=ctxu[64:65, :])
                rec_sp = norm_pool.tile(
                    [128, HQ // 128], F32, tag="recsp", name="rec_sp"
                )
                nc.vector.reciprocal(rec_sp, den_sp)
                rec_d = dram_pool.tile([HQ], F32, tag="recd", name="rec_d")
                nc.sync.dma_start(out=rec_d, in_=rec_sp)
                recb = norm_pool.tile([64, HQ], F32, tag="recb", name="recb")
                rec_bcast = bass.AP(
                    tensor=rec_d.tensor, offset=rec_d.offset,
                    ap=[[0, 64]] + list(rec_d.ap),
                )
                nc.sync.dma_start(out=recb, in_=rec_bcast)
                nc.vector.tensor_mul(
                    ctx_all[po : po + 64, p, Q0:Q1], ctxu[0:64, :], recb
                )
                yield

        def spacer(n):
            for _ in range(n):
                yield

        def warm_fill(n):
            """Dense dummy matmuls to hold the HAM clock-gate open while
            PE waits on DMA-latency chains (tail).  Singles only."""
            for _ in range(n):
                fil = _fil_tile("wf_ps")
                for _ in range(4):
                    nc.tensor.matmul(
                        fil[:, 0:128], lhsT=warm_sb, rhs=warm_sb,
                        start=True, stop=True,
                    )
                yield

        # ---- the schedule ----
        # Era windows, each paced by one scores stream; foreign work rides
        # the ACT-bound slack between score chunks:
        #   W1: scores(a00)   + proj ch2/ch3
        #   W2: scores(a10)   + vproj 0..15
        #   W3: scores(a01)   + AV(a00), AV(a10), outproj(0)
        #   W4: scores(a11)   + AV(a01), then AV(a11) pipelined 2-behind
        #   tail: AV(a11) rest, outproj(1)
        _DONE = object()

        def adv(g):
            return next(g, _DONE) is not _DONE

        def drive(sc, fills):
            """Drain sc; after each chunk emit `rate` units (fractional,
            carried) from the ordered fill list of (gen, rate)."""
            carry = 0.0
            for _ in sc:
                carry += 1.0
                while carry > 0 and fills:
                    g, rate = fills[0]
                    if not adv(g):
                        fills.pop(0)
                        continue
                    carry -= 1.0 / rate
            for g, _ in fills:  # drain leftovers
                for _ in g:
                    pass

        # pre-era: only what scores(a00) j0 chunk 0 needs (q 0..511 of
        # et0 and k-tile 0 of et2, both in x^T chunk 0)
        for _ in proj_qk_chunk(0, (0, 2)):
            pass

        pts00, pts10, pts01, pts11 = [], [], [], []
        # W1: 12 chunks of scores vs 12 proj units (ch1 e-tiles first --
        # j0's second chunk and k-tiles 4..7 need them)
        drive(scores(0, 0, pts00),
              [(proj_qk_chunk(1, (0, 2)), 1),
               (proj_qk_chunk(0, (1, 3)), 1), (proj_qk_chunk(1, (1, 3)), 1),
               (proj_qk_chunk(2, (0, 2, 1, 3)), 1),
               (proj_qk_chunk(3, (0, 2)), 1)])
        # W2: 12 chunks vs 18 units
        drive(scores(1, 0, pts10),
              [(proj_qk_chunk(3, (1, 3)), 1.5), (proj_v(0, 16), 1.5)])
        # W3: 27 chunks vs 10+10 AV units + 8 outproj (sequential drain:
        # each AV phase fully closes its ctx groups before the next opens;
        # outproj only runs once no ctx group is open, and the spacer gives
        # the a10 normalize DMA chain time to land first)
        drive(scores(0, 1, pts01),
              [(av(0, 0, pts00), 2), (av(1, 0, pts10), 2),
               (spacer(4), 1), (outproj(0, 8), 1)])
        # W4: 27 chunks vs 18 AV(a01) units, then AV(a11) trails behind its
        # own exp stream (strictly after av01's ctx groups close).
        av01 = av(0, 1, pts01)
        av11 = av(1, 1, pts11, early=causal)
        av01_done = False
        av11_n = 0
        for _ in scores(1, 1, pts11):
            b = 1.0
            while b > 0:
                if not av01_done:
                    if not adv(av01):
                        av01_done = True
                        continue
                    b -= 1.0
                elif av11_n < len(pts11):
                    if not adv(av11):
                        break
                    av11_n += 1
                    b -= 0.5
                else:
                    break
        nj11 = len(jrange(1)[0])
        while av11_n < nj11:
            if not adv(av11):
                break
            av11_n += 1
        if causal:
            # s-tiles 8..11 need only ctx chunk 0 (already normalized);
            # they run while chunk 1's reciprocal chain is in flight
            for _ in outproj(8, 12, split_cast=True):
                pass
            for _ in av11:   # chunk 1 evac + chain + normalize
                pass
            for _ in warm_fill(4):
                pass
            for _ in outproj(12, 16, split_cast=True):
                pass
        else:
            for _ in av11:
                pass
            for _ in warm_fill(10):
                pass
            for _ in outproj(8, 16, split_cast=False):
                pass


def _get_prog(causal: bool, nd: int):
    key = (causal, nd)
    if key not in _prog_cache:
        nc = bacc.Bacc("TRN2", target_bir_lowering=False, debug=False)
        xt_h = nc.dram_tensor("xt", [128, 4, nd, 512], BF16, kind="ExternalInput")
        w_h = nc.dram_tensor("w", [128, nd, E], BF16, kind="ExternalInput")
        wo_h = nc.dram_tensor("wo", [128, 2, D], BF16, kind="ExternalInput")
        out_h = nc.dram_tensor("out", [S, D], F16, kind="ExternalOutput")
        with tile.TileContext(nc) as tc:
            _emit(tc, xt_h, w_h, wo_h, out_h, causal, nd)
        nc.compile()
        _prog_cache[key] = nc
    return _prog_cache[key]


def _numpy_fallback(x, mask, qkv_w, qkv_b, out_w, out_b):
    qkv = x.reshape(B * S, D) @ qkv_w + qkv_b
    qkv = qkv.reshape(B, S, 3, H, DH)
    q, k, v = qkv[:, :, 0], qkv[:, :, 1], qkv[:, :, 2]
    sc = np.einsum("bqhd,bkhd->bhqk", q, k) / np.sqrt(np.float32(DH))
    sc = np.where(mask, sc, np.float32(-1e9))
    sc = sc - sc.max(-1, keepdims=True)
    a = np.exp(sc)
    a = a / a.sum(-1, keepdims=True)
    ctx = np.einsum("bhqk,bkhd->bqhd", a, v).reshape(B, S, D)
    return (ctx.reshape(B * S, D) @ out_w + out_b).reshape(B, S, D).astype(np.float32)


def kernel(x, mask, qkv_w, qkv_b, out_w, out_b):
    global last_results
    x = np.asarray(x, dtype=np.float32)
    mask = np.asarray(mask).astype(bool)
    qkv_w = np.asarray(qkv_w, dtype=np.float32)
    qkv_b = np.asarray(qkv_b, dtype=np.float32)
    out_w = np.asarray(out_w, dtype=np.float32)
    out_b = np.asarray(out_b, dtype=np.float32)

    m2 = mask.reshape(S, S)
    if m2.all():
        causal = False
    elif np.array_equal(m2, np.tril(np.ones((S, S), dtype=bool))):
        causal = True
    else:
        return _numpy_fallback(x, mask, qkv_w, qkv_b, out_w, out_b)

    has_b = bool(np.any(qkv_b))
    dd = D + 1 if has_b else D
    nd = (dd + 127) // 128
    nc = _get_prog(causal, nd)

    in_maps = []
    for c in range(NCORES):
        b, hg = divmod(c, 4)
        hs = hg * HPC
        cols = slice(hs * DH, (hs + HPC) * DH)
        wc = np.concatenate(
            [qkv_w[:, cols], qkv_w[:, D:][:, cols], qkv_w[:, 2 * D :][:, cols]], axis=1
        )
        xtc = x[b].T
        if has_b:
            bc = np.concatenate(
                [qkv_b[cols], qkv_b[D:][cols], qkv_b[2 * D :][cols]]
            )
            wc = np.concatenate([wc, bc[None, :]], axis=0)
            xtc = np.concatenate([xtc, np.ones((1, S), np.float32)], axis=0)
        # zero-pad contraction dim to nd*128 and retile to [128, ...]
        pad = nd * 128 - xtc.shape[0]
        if pad:
            xtc = np.concatenate([xtc, np.zeros((pad, S), np.float32)], axis=0)
            wc = np.concatenate([wc, np.zeros((pad, E), np.float32)], axis=0)
        # xt: [dsub*128+p, ch*512+c] -> [p, ch, dsub, c]
        xt4 = np.ascontiguousarray(
            xtc.reshape(nd, 128, 4, 512).transpose(1, 2, 0, 3)
        ).astype(NP_BF16)
        w3 = np.ascontiguousarray(
            wc.reshape(nd, 128, E).transpose(1, 0, 2)
        ).astype(NP_BF16)
        wo3 = np.ascontiguousarray(
            out_w[cols, :].reshape(2, 128, D).transpose(1, 0, 2)
        ).astype(NP_BF16)
        in_maps.append({"xt": xt4, "w": w3, "wo": wo3})

    trace = os.environ.get("KERNEL_TRACE", "0") == "1"
    last_results = run_bass_kernel_spmd(
        nc, in_maps, core_ids=list(range(NCORES)), trace=trace
    )
    out = np.zeros((B, S, D), dtype=np.float32)
    for c in range(NCORES):
        out[c // 4] += last_results.results[c]["out"].astype(np.float32)
    out += out_b[None, None, :]
    return out


# revision 29
# speedup vs baseline: 1.1487x; 1.1487x over previous
"""Multi-head attention (B=2, S=2048, D=1024, H=16) on 8 NeuronCores.

Sharding: core c -> (batch b = c//4, head-group hg = c%4 of 4 heads).
Each core: QKV projection for its 4 heads (bf16, f32 PSUM), transposed-score
flash attention (softmax denominator via an appended ones-column on V), and
the out-projection restricted to its heads' rows of out_w.  Host sums the 4
per-head-group fp16 partials per batch and adds out_b (exact, linear).

v3 schedule: the kernel is a single interleaved era paced by the ScalarE
exp stream (the true floor at ~80us/core).  Attention is split into a
scores+exp generator and an AV generator per (pair, q-half); large PE
blocks (late projection chunks, V-projection, out-projection) are emitted
between score chunks -- never inside an open PSUM accumulation group, and
out-projection only while no ctx group is open (mixing matmul groups into
open groups corrupts PSUM on HW -- measured).  Each half's AV phase rides
inside the next half's score phase, where its own scores+exp singles are
the (proven-safe) foreign work.

Other v3/v2 mechanics: host-retiled inputs -> one large DMA per tensor;
score matmuls head-interleaved at row-groups 0/64 (concurrent halves of
the PE array, keeps the HAM clock-gate open); exp merged over both heads
per 512-col chunk; fp16 partial outputs (host accumulates in f32).
"""

import os
import sys

sys.path.insert(0, "/opt/trn_rl_repo")

import numpy as np
import ml_dtypes

import concourse.bass as bass  # noqa: F401  (AP helpers)
import concourse.mybir as mybir
import concourse.tile as tile
from concourse import bacc
from concourse.bass_utils import run_bass_kernel_spmd
from concourse.masks import make_upper_triangular

B, S, D, H, DH = 2, 2048, 1024, 16, 64
NCORES = 8
HPC = 4            # heads per core
EQ = HPC * DH      # 256: q (or k, or v) columns per core
E = 3 * EQ         # 768: total projected columns per core
HQ = S // 2        # 1024: columns per q-half
BF16 = mybir.dt.bfloat16
F16 = mybir.dt.float16
F32 = mybir.dt.float32
NP_BF16 = ml_dtypes.bfloat16
EXPFN = mybir.ActivationFunctionType.Exp

_prog_cache: dict = {}
last_results = None  # BassKernelResults of the most recent run (for test.py)


def _emit(tc, xt_h, w_h, wo_h, out_h, causal, nd):
    nc = tc.nc

    with (
        tc.tile_pool(name="persist", bufs=1) as pp,
        tc.tile_pool(name="pt", bufs=17) as pt_pool,
        tc.tile_pool(name="norm", bufs=2) as norm_pool,
        tc.tile_pool(name="outsb", bufs=2) as out_pool,
        tc.tile_pool(name="dram", bufs=2, space="DRAM") as dram_pool,
        tc.tile_pool(name="ctxp", bufs=1, space="PSUM") as ctxp,
        tc.tile_pool(name="stp", bufs=2, space="PSUM") as stp,
    ):
        # ---- persistent SBUF tensors ----
        xt_sb = pp.tile([128, 4, nd, 512], BF16, tag="xt", name="xt_sb")
        w_sb = pp.tile([128, nd, E], BF16, tag="w", name="w_sb")
        wo_sb = pp.tile([128, 2, D], BF16, tag="wo", name="wo_sb")
        qkT_sb = pp.tile([128, 4, S], BF16, tag="qkT", name="qkT_sb")
        # V' per (k-tile j, head h): [128, 65], col 64 = ones (softmax denom)
        vp_sb = pp.tile([128, 16, HPC, 65], BF16, tag="vp", name="vp_sb")
        ctx_all = pp.tile([128, 2, S], BF16, tag="ctx", name="ctx_all")

        # one large DMA per tensor: 128 long contiguous partition lines
        nc.sync.dma_start(out=w_sb, in_=w_h[:, :, :])
        for ch in range(4):
            nc.sync.dma_start(out=xt_sb[:, ch], in_=xt_h[:, ch, :, :])
        nc.sync.dma_start(out=wo_sb, in_=wo_h[:, :, :])

        if causal:
            tri_f = pp.tile([128, 128], F32, tag="trif", name="tri_f")
            make_upper_triangular(nc, tri_f, val=1.0, diag=True)
            tri_bf = pp.tile([128, 128], BF16, tag="trib", name="tri_bf")
            nc.vector.tensor_copy(tri_bf, tri_f)

        nc.vector.memset(vp_sb[:, :, :, 64:65], 1.0)

        # Pre-warm the PE clock gate during the input-DMA window.
        warm_sb = pp.tile([128, 128], BF16, tag="warm", name="warm_sb")
        nc.vector.memset(warm_sb, 0.0)
        warm_ps = stp.tile([128, 2, 512], F32, tag="st", name="warm_ps")
        for _ in range(24):
            nc.tensor.matmul(
                warm_ps[:, 0, 0:128], lhsT=warm_sb, rhs=warm_sb,
                start=True, stop=True,
            )

        # ---- fill-work generators (complete matmul groups per yield) ----

        # The projection fill units borrow the ctx0/ctx1 PSUM allocations
        # (the only way to stay within 8 banks: st 2x2 + ctx 2x2). They are
        # fully drained before any AV phase opens ctx accumulation groups.
        _filln = [0]

        def _fil_tile(name):
            t = _filln[0] % 2
            _filln[0] += 1
            return ctxp.tile([128, HQ], F32, tag=f"ctx{t}", name=name)

        def proj_qk_chunk(ch, ets=(0, 1, 2, 3)):
            """Q^T/K^T projection for one 512-col s-chunk, one e-tile/yield."""
            for et in ets:
                fil = _fil_tile("qk_ps")
                ps = fil[:, 0:512]
                for d in range(nd):
                    nc.tensor.matmul(
                        ps,
                        lhsT=w_sb[:, d, 128 * et : 128 * (et + 1)],
                        rhs=xt_sb[:, ch, d, :],
                        start=(d == 0),
                        stop=(d == nd - 1),
                    )
                nc.vector.tensor_copy(
                    qkT_sb[:, et, 512 * ch : 512 * (ch + 1)], ps
                )
                yield

        def proj_v(lo, hi):
            """V in [s, e_v] layout for all 4 heads; one s-tile per yield."""
            for i in range(lo, hi):
                ch, io = i // 4, i % 4
                fil = _fil_tile("v_ps")
                vps = fil[:, 0:256]
                for d in range(nd):
                    nc.tensor.matmul(
                        vps,
                        lhsT=xt_sb[:, ch, d, 128 * io : 128 * (io + 1)],
                        rhs=w_sb[:, d, 2 * EQ : 3 * EQ],
                        start=(d == 0),
                        stop=(d == nd - 1),
                    )
                nc.vector.tensor_copy(
                    vp_sb[:, i, :, 0:64],
                    vps.rearrange("p (h e) -> p h e", h=HPC),
                )
                yield

        def outproj(lo, hi, split_cast=False):
            """Out-projection + fp16 store for s-tiles [lo, hi).

            Must only run while NO ctx accumulation group is open.
            """
            for i in range(lo, hi):
                ops = stp.tile([128, 2, 512], F32, tag="st", name="o_ps")
                for et in range(2):
                    for c in range(2):
                        nc.tensor.matmul(
                            ops[:, c, :],
                            lhsT=ctx_all[:, et, 128 * i : 128 * (i + 1)],
                            rhs=wo_sb[:, et, 512 * c : 512 * (c + 1)],
                            start=(et == 0),
                            stop=(et == 1),
                        )
                osb = out_pool.tile([128, 2, 512], F16, tag="osb", name="o_sb")
                if split_cast:
                    nc.vector.tensor_copy(osb[:, 0, :], ops[:, 0, :])
                    nc.scalar.copy(osb[:, 1, :], ops[:, 1, :])
                else:
                    nc.vector.tensor_copy(osb, ops)
                nc.sync.dma_start(
                    out=out_h[128 * i : 128 * (i + 1), :],
                    in_=osb.rearrange("p a b -> p (a b)"),
                )
                yield

        # ---- attention, split into scores+exp and AV generators ----

        def jrange(half):
            Q0, Q1 = HQ * half, HQ * (half + 1)
            return [j for j in range(16) if not (causal and 128 * j >= Q1)], Q0, Q1

        def scores(p, half, pts_out):
            """Scores+exp for all j of (pair, half); yields per 512-chunk.

            Emits only single (start=stop=True) matmuls + ACT/gpsimd work,
            so it is safe to interleave with anything.
            Appends (j, pts) to pts_out as each j's exp completes.
            """
            qsl = [qkT_sb[hh * 64 : hh * 64 + 64, p, :] for hh in range(2)]
            ksl = [qkT_sb[hh * 64 : hh * 64 + 64, 2 + p, :] for hh in range(2)]
            js, Q0, Q1 = jrange(half)
            for j in js:
                q0 = max(128 * j, Q0) if causal else Q0
                qcols = Q1 - q0
                pts = pt_pool.tile([128, 2, HQ], BF16, tag="pt", name="pt")
                off = 0
                while off < qcols:
                    clen = min(512, qcols - off)
                    st = stp.tile([128, 2, 512], F32, tag="st", name="st_ps")
                    for hh in range(2):
                        nc.tensor.matmul(
                            st[:, hh, 0:clen],
                            lhsT=ksl[hh][:, 128 * j : 128 * (j + 1)],
                            rhs=qsl[hh][:, q0 + off : q0 + off + clen],
                            start=True,
                            stop=True,
                        )
                    nc.scalar.activation(
                        pts[:, :, off : off + clen],
                        st[:, :, 0:clen],
                        EXPFN,
                        scale=0.125,
                    )
                    off += clen
                    if off < qcols:
                        yield (352 + 2 * clen) / 1.2
                if causal and 128 * j >= Q0:
                    for hh in range(2):
                        nc.gpsimd.tensor_mul(
                            pts[:, hh, 0:128], pts[:, hh, 0:128], tri_bf
                        )
                pts_out.append((j, pts))
                yield (352 + 2 * clen) / 1.2

        def _norm_chunk(p, hh, ctx_ps, Q0, c):
            """Evacuate + normalize one 512-col ctx chunk of one head."""
            po = hh * 64
            cs = slice(512 * c, 512 * (c + 1))
            ctxu = norm_pool.tile([65, 512], F32, tag=f"ecu{c}{hh}", name="ecu")
            nc.vector.tensor_copy(ctxu, ctx_ps[hh][0:65, cs])
            den_sp = norm_pool.tile([128, 4], F32, tag=f"eds{c}{hh}", name="eds")
            nc.sync.dma_start(out=den_sp, in_=ctxu[64:65, :])
            rec_sp = norm_pool.tile([128, 4], F32, tag=f"ers{c}{hh}", name="ers")
            nc.vector.reciprocal(rec_sp, den_sp)
            rec_d = dram_pool.tile([512], F32, tag=f"erd{c}{hh}", name="erd")
            nc.sync.dma_start(out=rec_d, in_=rec_sp)
            recb = norm_pool.tile([64, 512], F32, tag=f"erb{c}", name="erb")
            rec_bcast = bass.AP(
                tensor=rec_d.tensor, offset=rec_d.offset,
                ap=[[0, 64]] + list(rec_d.ap),
            )
            nc.sync.dma_start(out=recb, in_=rec_bcast)
            nc.vector.tensor_mul(
                ctx_all[po : po + 64, p, Q0 + 512 * c : Q0 + 512 * (c + 1)],
                ctxu[0:64, :], recb,
            )

        def av(p, half, pts_in, early=False):
            """AV + evac + normalize for (pair, half); consumes pts_in.

            Opens ctx accumulation groups: between this generator's first
            and last AV yield, only `scores` work may interleave.
            """
            js, Q0, Q1 = jrange(half)
            ctx_ps = [
                ctxp.tile([128, HQ], F32, tag=f"ctx{hh}", name=f"ctx_ps{hh}")[
                    0:65, :
                ]
                for hh in range(2)
            ]
            for idx in range(len(js)):
                if idx >= len(pts_in):
                    raise RuntimeError(f"av({p},{half}) starved: idx={idx} len={len(pts_in)}")
                j, pts = pts_in[idx]
                q0 = max(128 * j, Q0) if causal else Q0
                c0 = (q0 - Q0) // 512
                zlen = (q0 - Q0) - 512 * c0
                for hh in range(2):
                    for c in range(c0, 2):
                        gc = 2 * half + c
                        jl = min(15, 4 * gc + 3) if causal else 15
                        if c == c0:
                            dst = ctx_ps[hh][:, 512 * c0 + zlen : 512 * (c0 + 1)]
                            src = pts[:, hh, 0 : 512 - zlen]
                        else:
                            dst = ctx_ps[hh][:, 512 * c : 512 * (c + 1)]
                            src = pts[
                                :, hh,
                                512 * (c - c0) - zlen : 512 * (c - c0 + 1) - zlen,
                            ]
                        nc.tensor.matmul(
                            dst,
                            lhsT=vp_sb[:, j, 2 * p + hh, :],
                            rhs=src,
                            start=(j == 0),
                            stop=(j == jl),
                        )
                if early and causal and idx == 11:
                    # chunk 0 of this half closed at j=11: evacuate and
                    # start its reciprocal chain now, riding the last AVs
                    for hh in range(2):
                        _norm_chunk(p, hh, ctx_ps, Q0, 0)
                yield
            if early and causal:
                for hh in range(2):
                    _norm_chunk(p, hh, ctx_ps, Q0, 1)
                    yield
                return
            # Evacuate unnormalized ctx^T (+ denominator row 64) to SBUF,
            # normalize off-path via a DRAM-bounce reciprocal broadcast.
            for hh in range(2):
                po = hh * 64
                ctxu = norm_pool.tile(
                    [65, HQ], F32, tag=f"ctxu{hh}", name=f"ctxu{hh}"
                )
                nc.vector.tensor_copy(ctxu, ctx_ps[hh])
                den_sp = norm_pool.tile(
                    [128, HQ // 128], F32, tag="densp", name="den_sp"
                )
                nc.sync.dma_start(out=den_sp, in_=ctxu[64:65, :])
                rec_sp = norm_pool.tile(
                    [128, HQ // 128], F32, tag="recsp", name="rec_sp"
                )
                nc.vector.reciprocal(rec_sp, den_sp)
                rec_d = dram_pool.tile([HQ], F32, tag="recd", name="rec_d")
                nc.sync.dma_start(out=rec_d, in_=rec_sp)
                recb = norm_pool.tile([64, HQ], F32, tag="recb", name="recb")
                rec_bcast = bass.AP(
                    tensor=rec_d.tensor, offset=rec_d.offset,
                    ap=[[0, 64]] + list(rec_d.ap),
                )
                nc.sync.dma_start(out=recb, in_=rec_bcast)
                nc.vector.tensor_mul(
                    ctx_all[po : po + 64, p, Q0:Q1], ctxu[0:64, :], recb
                )
                yield

        def spacer(n):
            for _ in range(n):
                yield

        def warm_fill(n):
            """Dense dummy matmuls to hold the HAM clock-gate open while
            PE waits on DMA-latency chains (tail).  Singles only."""
            for _ in range(n):
                fil = _fil_tile("wf_ps")
                for _ in range(4):
                    nc.tensor.matmul(
                        fil[:, 0:128], lhsT=warm_sb, rhs=warm_sb,
                        start=True, stop=True,
                    )
                yield

        # ---- the schedule ----
        # Era windows, each paced by one scores stream; foreign work rides
        # the ACT-bound slack between score chunks:
        #   W1: scores(a00)   + proj ch2/ch3
        #   W2: scores(a10)   + vproj 0..15
        #   W3: scores(a01)   + AV(a00), AV(a10), outproj(0)
        #   W4: scores(a11)   + AV(a01), then AV(a11) pipelined 2-behind
        #   tail: AV(a11) rest, outproj(1)
        _DONE = object()

        def adv(g):
            return next(g, _DONE) is not _DONE

        def drive(sc, fills):
            """Drain sc; after each chunk emit `rate` units (fractional,
            carried) from the ordered fill list of (gen, rate)."""
            carry = 0.0
            for _ in sc:
                carry += 1.0
                while carry > 0 and fills:
                    g, rate = fills[0]
                    if not adv(g):
                        fills.pop(0)
                        continue
                    carry -= 1.0 / rate
            for g, _ in fills:  # drain leftovers
                for _ in g:
                    pass

        # pre-era: only what scores(a00) j0 chunk 0 needs (q 0..511 of
        # et0 and k-tile 0 of et2, both in x^T chunk 0)
        for _ in proj_qk_chunk(0, (0, 2)):
            pass

        pts00, pts10, pts01, pts11 = [], [], [], []
        # W1: 12 chunks of scores vs 12 proj units (ch1 e-tiles first --
        # j0's second chunk and k-tiles 4..7 need them)
        drive(scores(0, 0, pts00),
              [(proj_qk_chunk(1, (0, 2)), 1),
               (proj_qk_chunk(0, (1, 3)), 1), (proj_qk_chunk(1, (1, 3)), 1),
               (proj_qk_chunk(2, (0, 2, 1, 3)), 1),
               (proj_qk_chunk(3, (0, 2)), 1)])
        # W2: 12 chunks vs 18 units
        drive(scores(1, 0, pts10),
              [(proj_qk_chunk(3, (1, 3)), 1.5), (proj_v(0, 16), 1.5)])
        # W3: 27 chunks vs 10+10 AV units + 8 outproj (sequential drain:
        # each AV phase fully closes its ctx groups before the next opens;
        # outproj only runs once no ctx group is open, and the spacer gives
        # the a10 normalize DMA chain time to land first)
        drive(scores(0, 1, pts01),
              [(av(0, 0, pts00), 2), (av(1, 0, pts10), 2),
               (spacer(4), 1), (outproj(0, 8), 1)])
        # W4: 27 chunks vs 18 AV(a01) units, then AV(a11) trails behind its
        # own exp stream (strictly after av01's ctx groups close).
        av01 = av(0, 1, pts01)
        av11 = av(1, 1, pts11, early=causal)
        av01_done = False
        av11_n = 0
        for _ in scores(1, 1, pts11):
            b = 1.0
            while b > 0:
                if not av01_done:
                    if not adv(av01):
                        av01_done = True
                        continue
                    b -= 1.0
                elif av11_n < len(pts11):
                    if not adv(av11):
                        break
                    av11_n += 1
                    b -= 0.5
                else:
                    break
        nj11 = len(jrange(1)[0])
        while av11_n < nj11:
            if not adv(av11):
                break
            av11_n += 1
        if causal:
            # s-tiles 8..11 need only ctx chunk 0 (already normalized);
            # they run while chunk 1's reciprocal chain is in flight
            for _ in outproj(8, 12, split_cast=True):
                pass
            for _ in av11:   # chunk 1 evac + chain + normalize
                pass
            for _ in warm_fill(4):
                pass
            for _ in outproj(12, 16, split_cast=True):
                pass
        else:
            for _ in av11:
                pass
            for _ in warm_fill(10):
                pass
            for _ in outproj(8, 16, split_cast=False):
                pass


def _get_prog(causal: bool, nd: int):
    key = (causal, nd)
    if key not in _prog_cache:
        nc = bacc.Bacc("TRN2", target_bir_lowering=False, debug=False)
        xt_h = nc.dram_tensor("xt", [128, 4, nd, 512], BF16, kind="ExternalInput")
        w_h = nc.dram_tensor("w", [128, nd, E], BF16, kind="ExternalInput")
        wo_h = nc.dram_tensor("wo", [128, 2, D], BF16, kind="ExternalInput")
        out_h = nc.dram_tensor("out", [S, D], F16, kind="ExternalOutput")
        with tile.TileContext(nc) as tc:
            _emit(tc, xt_h, w_h, wo_h, out_h, causal, nd)
        nc.compile()
        _prog_cache[key] = nc
    return _prog_cache[key]


def _numpy_fallback(x, mask, qkv_w, qkv_b, out_w, out_b):
    qkv = x.reshape(B * S, D) @ qkv_w + qkv_b
    qkv = qkv.reshape(B, S, 3, H, DH)
    q, k, v = qkv[:, :, 0], qkv[:, :, 1], qkv[:, :, 2]
    sc = np.einsum("bqhd,bkhd->bhqk", q, k) / np.sqrt(np.float32(DH))
    sc = np.where(mask, sc, np.float32(-1e9))
    sc = sc - sc.max(-1, keepdims=True)
    a = np.exp(sc)
    a = a / a.sum(-1, keepdims=True)
    ctx = np.einsum("bhqk,bkhd->bqhd", a, v).reshape(B, S, D)
    return (ctx.reshape(B * S, D) @ out_w + out_b).reshape(B, S, D).astype(np.float32)


def kernel(x, mask, qkv_w, qkv_b, out_w, out_b):
    global last_results
    x = np.asarray(x, dtype=np.float32)
    mask = np.asarray(mask).astype(bool)
    qkv_w = np.asarray(qkv_w, dtype=np.float32)
    qkv_b = np.asarray(qkv_b, dtype=np.float32)
    out_w = np.asarray(out_w, dtype=np.float32)
    out_b = np.asarray(out_b, dtype=np.float32)

    m2 = mask.reshape(S, S)
    if m2.all():
        causal = False
    elif np.array_equal(m2, np.tril(np.ones((S, S), dtype=bool))):
        causal = True
    else:
        return _numpy_fallback(x, mask, qkv_w, qkv_b, out_w, out_b)

    has_b = bool(np.any(qkv_b))
    dd = D + 1 if has_b else D
    nd = (dd + 127) // 128
    nc = _get_prog(causal, nd)

    in_maps = []
    for c in range(NCORES):
        b, hg = divmod(c, 4)
        hs = hg * HPC
        cols = slice(hs * DH, (hs + HPC) * DH)
        wc = np.concatenate(
            [qkv_w[:, cols], qkv_w[:, D:][:, cols], qkv_w[:, 2 * D :][:, cols]], axis=1
        )
        xtc = x[b].T
        if has_b:
            bc = np.concatenate(
                [qkv_b[cols], qkv_b[D:][cols], qkv_b[2 * D :][cols]]
            )
            wc = np.concatenate([wc, bc[None, :]], axis=0)
            xtc = np.concatenate([xtc, np.ones((1, S), np.float32)], axis=0)
        # zero-pad contraction dim to nd*128 and retile to [128, ...]
        pad = nd * 128 - xtc.shape[0]
        if pad:
            xtc = np.concatenate([xtc, np.zeros((pad, S), np.float32)], axis=0)
            wc = np.concatenate([wc, np.zeros((pad, E), np.float32)], axis=0)
        # xt: [dsub*128+p, ch*512+c] -> [p, ch, dsub, c]
        xt4 = np.ascontiguousarray(
            xtc.reshape(nd, 128, 4, 512).transpose(1, 2, 0, 3)
        ).astype(NP_BF16)
        w3 = np.ascontiguousarray(
            wc.reshape(nd, 128, E).transpose(1, 0, 2)
        ).astype(NP_BF16)
        wo3 = np.ascontiguousarray(
            out_w[cols, :].reshape(2, 128, D).transpose(1, 0, 2)
        ).astype(NP_BF16)
        in_maps.append({"xt": xt4, "w": w3, "wo": wo3})

    trace = os.environ.get("KERNEL_TRACE", "0") == "1"
    last_results = run_bass_kernel_spmd(
        nc, in_maps, core_ids=list(range(NCORES)), trace=trace
    )
    out = np.zeros((B, S, D), dtype=np.float32)
    for c in range(NCORES):
        out[c // 4] += last_results.results[c]["out"].astype(np.float32)
    out += out_b[None, None, :]
    return out


# revision 30
# speedup vs baseline: 1.1851x; 1.0317x over previous
"""Multi-head attention (B=2, S=2048, D=1024, H=16) on 8 NeuronCores.

Sharding: core c -> (batch b = c//4, head-group hg = c%4 of 4 heads).
Each core: QKV projection for its 4 heads (bf16, f32 PSUM), transposed-score
flash attention (softmax denominator via an appended ones-column on V), and
the out-projection restricted to its heads' rows of out_w.  Host sums the 4
per-head-group fp16 partials per batch and adds out_b (exact, linear).

v3 schedule: the kernel is a single interleaved era paced by the ScalarE
exp stream (the true floor at ~80us/core).  Attention is split into a
scores+exp generator and an AV generator per (pair, q-half); large PE
blocks (late projection chunks, V-projection, out-projection) are emitted
between score chunks -- never inside an open PSUM accumulation group, and
out-projection only while no ctx group is open (mixing matmul groups into
open groups corrupts PSUM on HW -- measured).  Each half's AV phase rides
inside the next half's score phase, where its own scores+exp singles are
the (proven-safe) foreign work.

Other v3/v2 mechanics: host-retiled inputs -> one large DMA per tensor;
score matmuls head-interleaved at row-groups 0/64 (concurrent halves of
the PE array, keeps the HAM clock-gate open); exp merged over both heads
per 512-col chunk; fp16 partial outputs (host accumulates in f32).
"""

import os
import sys

sys.path.insert(0, "/opt/trn_rl_repo")

import numpy as np
import ml_dtypes

import concourse.bass as bass  # noqa: F401  (AP helpers)
import concourse.mybir as mybir
import concourse.tile as tile
from concourse import bacc
from concourse.bass_utils import run_bass_kernel_spmd
from concourse.masks import make_upper_triangular

B, S, D, H, DH = 2, 2048, 1024, 16, 64
NCORES = 8
HPC = 4            # heads per core
EQ = HPC * DH      # 256: q (or k, or v) columns per core
E = 3 * EQ         # 768: total projected columns per core
HQ = S // 2        # 1024: columns per q-half
BF16 = mybir.dt.bfloat16
F16 = mybir.dt.float16
F32 = mybir.dt.float32
NP_BF16 = ml_dtypes.bfloat16
EXPFN = mybir.ActivationFunctionType.Exp

_prog_cache: dict = {}
last_results = None  # BassKernelResults of the most recent run (for test.py)


def _emit(tc, xt_h, w_h, wo_h, out_h, causal, nd):
    nc = tc.nc

    with (
        tc.tile_pool(name="persist", bufs=1) as pp,
        tc.tile_pool(name="pt", bufs=20) as pt_pool,
        tc.tile_pool(name="norm", bufs=2) as norm_pool,
        tc.tile_pool(name="outsb", bufs=3) as out_pool,
        tc.tile_pool(name="dram", bufs=2, space="DRAM") as dram_pool,
        tc.tile_pool(name="ctxp", bufs=1, space="PSUM") as ctxp,
        tc.tile_pool(name="stp", bufs=2, space="PSUM") as stp,
    ):
        # ---- persistent SBUF tensors ----
        xt_sb = pp.tile([128, 4, nd, 512], BF16, tag="xt", name="xt_sb")
        w_sb = pp.tile([128, nd, E], BF16, tag="w", name="w_sb")
        wo_sb = pp.tile([128, 2, D], BF16, tag="wo", name="wo_sb")
        qkT_sb = pp.tile([128, 4, S], BF16, tag="qkT", name="qkT_sb")
        # V' per (k-tile j, head h): [128, 65], col 64 = ones (softmax denom)
        vp_sb = pp.tile([128, 16, HPC, 65], BF16, tag="vp", name="vp_sb")
        ctx_all = pp.tile([128, 2, S], BF16, tag="ctx", name="ctx_all")

        # one large DMA per tensor: 128 long contiguous partition lines
        nc.sync.dma_start(out=w_sb, in_=w_h[:, :, :])
        for ch in range(4):
            nc.sync.dma_start(out=xt_sb[:, ch], in_=xt_h[:, ch, :, :])
        nc.sync.dma_start(out=wo_sb, in_=wo_h[:, :, :])

        if causal:
            tri_f = pp.tile([128, 128], F32, tag="trif", name="tri_f")
            make_upper_triangular(nc, tri_f, val=1.0, diag=True)
            tri_bf = pp.tile([128, 128], BF16, tag="trib", name="tri_bf")
            nc.vector.tensor_copy(tri_bf, tri_f)

        nc.vector.memset(vp_sb[:, :, :, 64:65], 1.0)

        # Pre-warm the PE clock gate during the input-DMA window.
        warm_sb = pp.tile([128, 128], BF16, tag="warm", name="warm_sb")
        nc.vector.memset(warm_sb, 0.0)
        warm_ps = stp.tile([128, 2, 512], F32, tag="st", name="warm_ps")
        for _ in range(24):
            nc.tensor.matmul(
                warm_ps[:, 0, 0:128], lhsT=warm_sb, rhs=warm_sb,
                start=True, stop=True,
            )

        # ---- fill-work generators (complete matmul groups per yield) ----

        # The projection fill units borrow the ctx0/ctx1 PSUM allocations
        # (the only way to stay within 8 banks: st 2x2 + ctx 2x2). They are
        # fully drained before any AV phase opens ctx accumulation groups.
        _filln = [0]

        def _fil_tile(name):
            t = _filln[0] % 2
            _filln[0] += 1
            return ctxp.tile([128, HQ], F32, tag=f"ctx{t}", name=name)

        def proj_qk_chunk(ch, ets=(0, 1, 2, 3)):
            """Q^T/K^T projection for one 512-col s-chunk, one e-tile/yield."""
            for et in ets:
                fil = _fil_tile("qk_ps")
                ps = fil[:, 0:512]
                for d in range(nd):
                    nc.tensor.matmul(
                        ps,
                        lhsT=w_sb[:, d, 128 * et : 128 * (et + 1)],
                        rhs=xt_sb[:, ch, d, :],
                        start=(d == 0),
                        stop=(d == nd - 1),
                    )
                nc.vector.tensor_copy(
                    qkT_sb[:, et, 512 * ch : 512 * (ch + 1)], ps
                )
                yield

        def proj_v(lo, hi):
            """V in [s, e_v] layout for all 4 heads; one s-tile per yield."""
            for i in range(lo, hi):
                ch, io = i // 4, i % 4
                fil = _fil_tile("v_ps")
                vps = fil[:, 0:256]
                for d in range(nd):
                    nc.tensor.matmul(
                        vps,
                        lhsT=xt_sb[:, ch, d, 128 * io : 128 * (io + 1)],
                        rhs=w_sb[:, d, 2 * EQ : 3 * EQ],
                        start=(d == 0),
                        stop=(d == nd - 1),
                    )
                nc.vector.tensor_copy(
                    vp_sb[:, i, :, 0:64],
                    vps.rearrange("p (h e) -> p h e", h=HPC),
                )
                yield

        def outproj(half, split_cast=False):
            """Out-projection + fp16 store for one q-half, one s-tile/yield.

            Must only run while NO ctx accumulation group is open.
            """
            for i in range(8 * half, 8 * half + 8):
                ops = stp.tile([128, 2, 512], F32, tag="st", name="o_ps")
                for et in range(2):
                    for c in range(2):
                        nc.tensor.matmul(
                            ops[:, c, :],
                            lhsT=ctx_all[:, et, 128 * i : 128 * (i + 1)],
                            rhs=wo_sb[:, et, 512 * c : 512 * (c + 1)],
                            start=(et == 0),
                            stop=(et == 1),
                        )
                osb = out_pool.tile([128, 2, 512], F16, tag="osb", name="o_sb")
                if split_cast:
                    nc.vector.tensor_copy(osb[:, 0, :], ops[:, 0, :])
                    nc.scalar.copy(osb[:, 1, :], ops[:, 1, :])
                else:
                    nc.vector.tensor_copy(osb, ops)
                nc.sync.dma_start(
                    out=out_h[128 * i : 128 * (i + 1), :],
                    in_=osb.rearrange("p a b -> p (a b)"),
                )
                yield

        # ---- attention, split into scores+exp and AV generators ----

        def jrange(half):
            Q0, Q1 = HQ * half, HQ * (half + 1)
            return [j for j in range(16) if not (causal and 128 * j >= Q1)], Q0, Q1

        def scores(p, half, pts_out):
            """Scores+exp for all j of (pair, half); yields per 512-chunk.

            Emits only single (start=stop=True) matmuls + ACT/gpsimd work,
            so it is safe to interleave with anything.
            Appends (j, pts) to pts_out as each j's exp completes.
            """
            qsl = [qkT_sb[hh * 64 : hh * 64 + 64, p, :] for hh in range(2)]
            ksl = [qkT_sb[hh * 64 : hh * 64 + 64, 2 + p, :] for hh in range(2)]
            js, Q0, Q1 = jrange(half)
            for j in js:
                q0 = max(128 * j, Q0) if causal else Q0
                qcols = Q1 - q0
                pts = pt_pool.tile([128, 2, HQ], BF16, tag="pt", name="pt")
                off = 0
                while off < qcols:
                    clen = min(512, qcols - off)
                    st = stp.tile([128, 2, 512], F32, tag="st", name="st_ps")
                    for hh in range(2):
                        nc.tensor.matmul(
                            st[:, hh, 0:clen],
                            lhsT=ksl[hh][:, 128 * j : 128 * (j + 1)],
                            rhs=qsl[hh][:, q0 + off : q0 + off + clen],
                            start=True,
                            stop=True,
                        )
                    nc.scalar.activation(
                        pts[:, :, off : off + clen],
                        st[:, :, 0:clen],
                        EXPFN,
                        scale=0.125,
                    )
                    off += clen
                    if off < qcols:
                        yield (352 + 2 * clen) / 1.2
                if causal and 128 * j >= Q0:
                    for hh in range(2):
                        nc.gpsimd.tensor_mul(
                            pts[:, hh, 0:128], pts[:, hh, 0:128], tri_bf
                        )
                pts_out.append((j, pts))
                yield (352 + 2 * clen) / 1.2

        def av(p, half, pts_in):
            """AV + evac + normalize for (pair, half); consumes pts_in.

            Opens ctx accumulation groups: between this generator's first
            and last AV yield, only `scores` work may interleave.
            """
            js, Q0, Q1 = jrange(half)
            ctx_ps = [
                ctxp.tile([128, HQ], F32, tag=f"ctx{hh}", name=f"ctx_ps{hh}")[
                    0:65, :
                ]
                for hh in range(2)
            ]
            for idx in range(len(js)):
                if idx >= len(pts_in):
                    raise RuntimeError(f"av({p},{half}) starved: idx={idx} len={len(pts_in)}")
                j, pts = pts_in[idx]
                q0 = max(128 * j, Q0) if causal else Q0
                c0 = (q0 - Q0) // 512
                zlen = (q0 - Q0) - 512 * c0
                for hh in range(2):
                    for c in range(c0, 2):
                        gc = 2 * half + c
                        jl = min(15, 4 * gc + 3) if causal else 15
                        if c == c0:
                            dst = ctx_ps[hh][:, 512 * c0 + zlen : 512 * (c0 + 1)]
                            src = pts[:, hh, 0 : 512 - zlen]
                        else:
                            dst = ctx_ps[hh][:, 512 * c : 512 * (c + 1)]
                            src = pts[
                                :, hh,
                                512 * (c - c0) - zlen : 512 * (c - c0 + 1) - zlen,
                            ]
                        nc.tensor.matmul(
                            dst,
                            lhsT=vp_sb[:, j, 2 * p + hh, :],
                            rhs=src,
                            start=(j == 0),
                            stop=(j == jl),
                        )
                yield
            # Evacuate unnormalized ctx^T (+ denominator row 64) to SBUF,
            # normalize off-path via a DRAM-bounce reciprocal broadcast.
            for hh in range(2):
                po = hh * 64
                ctxu = norm_pool.tile(
                    [65, HQ], F32, tag=f"ctxu{hh}", name=f"ctxu{hh}"
                )
                nc.vector.tensor_copy(ctxu, ctx_ps[hh])
                den_sp = norm_pool.tile(
                    [128, HQ // 128], F32, tag="densp", name="den_sp"
                )
                nc.sync.dma_start(out=den_sp, in_=ctxu[64:65, :])
                rec_sp = norm_pool.tile(
                    [128, HQ // 128], F32, tag="recsp", name="rec_sp"
                )
                nc.vector.reciprocal(rec_sp, den_sp)
                rec_d = dram_pool.tile([HQ], F32, tag="recd", name="rec_d")
                nc.sync.dma_start(out=rec_d, in_=rec_sp)
                recb = norm_pool.tile([64, HQ], F32, tag="recb", name="recb")
                rec_bcast = bass.AP(
                    tensor=rec_d.tensor, offset=rec_d.offset,
                    ap=[[0, 64]] + list(rec_d.ap),
                )
                nc.sync.dma_start(out=recb, in_=rec_bcast)
                nc.vector.tensor_mul(
                    ctx_all[po : po + 64, p, Q0:Q1], ctxu[0:64, :], recb
                )
                yield

        def spacer(n):
            for _ in range(n):
                yield

        def warm_fill(n):
            """Dense dummy matmuls to hold the HAM clock-gate open while
            PE waits on DMA-latency chains (tail).  Singles only."""
            for _ in range(n):
                fil = _fil_tile("wf_ps")
                for _ in range(4):
                    nc.tensor.matmul(
                        fil[:, 0:128], lhsT=warm_sb, rhs=warm_sb,
                        start=True, stop=True,
                    )
                yield

        # ---- the schedule ----
        # Era windows, each paced by one scores stream; foreign work rides
        # the ACT-bound slack between score chunks:
        #   W1: scores(a00)   + proj ch2/ch3
        #   W2: scores(a10)   + vproj 0..15
        #   W3: scores(a01)   + AV(a00), AV(a10), outproj(0)
        #   W4: scores(a11)   + AV(a01), then AV(a11) pipelined 2-behind
        #   tail: AV(a11) rest, outproj(1)
        _DONE = object()

        def adv(g):
            return next(g, _DONE) is not _DONE

        def drive(sc, fills):
            """Drain sc; after each chunk emit `rate` units (fractional,
            carried) from the ordered fill list of (gen, rate)."""
            carry = 0.0
            for _ in sc:
                carry += 1.0
                while carry > 0 and fills:
                    g, rate = fills[0]
                    if not adv(g):
                        fills.pop(0)
                        continue
                    carry -= 1.0 / rate
            for g, _ in fills:  # drain leftovers
                for _ in g:
                    pass

        # pre-era: only what scores(a00) j0 chunk 0 needs (q 0..511 of
        # et0 and k-tile 0 of et2, both in x^T chunk 0)
        for _ in proj_qk_chunk(0, (0, 2)):
            pass

        pts00, pts10, pts01, pts11 = [], [], [], []
        # W1: 12 chunks of scores vs 12 proj units (ch1 e-tiles first --
        # j0's second chunk and k-tiles 4..7 need them)
        drive(scores(0, 0, pts00),
              [(proj_qk_chunk(1, (0, 2)), 1),
               (proj_qk_chunk(0, (1, 3)), 1), (proj_qk_chunk(1, (1, 3)), 1),
               (proj_qk_chunk(2, (0, 2, 1, 3)), 1),
               (proj_qk_chunk(3, (0, 2)), 1)])
        # W2: 12 chunks vs 18 units
        drive(scores(1, 0, pts10),
              [(proj_qk_chunk(3, (1, 3)), 1.5), (proj_v(0, 16), 1.5)])
        # W3: 27 chunks vs 10+10 AV units + 8 outproj (sequential drain:
        # each AV phase fully closes its ctx groups before the next opens;
        # outproj only runs once no ctx group is open, and the spacer gives
        # the a10 normalize DMA chain time to land first)
        drive(scores(0, 1, pts01),
              [(av(0, 0, pts00), 2), (av(1, 0, pts10), 2),
               (spacer(4), 1), (outproj(0), 1)])
        # W4: 27 chunks vs 18 AV(a01) units, then AV(a11) trails behind its
        # own exp stream (strictly after av01's ctx groups close).
        av01 = av(0, 1, pts01)
        av11 = av(1, 1, pts11)
        av01_done = False
        av11_n = 0
        for _ in scores(1, 1, pts11):
            b = 1.0
            while b > 0:
                if not av01_done:
                    if not adv(av01):
                        av01_done = True
                        continue
                    b -= 1.0
                elif av11_n < len(pts11):
                    if not adv(av11):
                        break
                    av11_n += 1
                    b -= 0.5
                else:
                    break
        for _ in av11:
            pass
        for _ in warm_fill(10):
            pass
        for _ in outproj(1, split_cast=False):
            pass


def _get_prog(causal: bool, nd: int):
    key = (causal, nd)
    if key not in _prog_cache:
        nc = bacc.Bacc("TRN2", target_bir_lowering=False, debug=False)
        xt_h = nc.dram_tensor("xt", [128, 4, nd, 512], BF16, kind="ExternalInput")
        w_h = nc.dram_tensor("w", [128, nd, E], BF16, kind="ExternalInput")
        wo_h = nc.dram_tensor("wo", [128, 2, D], BF16, kind="ExternalInput")
        out_h = nc.dram_tensor("out", [S, D], F16, kind="ExternalOutput")
        with tile.TileContext(nc) as tc:
            _emit(tc, xt_h, w_h, wo_h, out_h, causal, nd)
        nc.compile()
        _prog_cache[key] = nc
    return _prog_cache[key]


def _numpy_fallback(x, mask, qkv_w, qkv_b, out_w, out_b):
    qkv = x.reshape(B * S, D) @ qkv_w + qkv_b
    qkv = qkv.reshape(B, S, 3, H, DH)
    q, k, v = qkv[:, :, 0], qkv[:, :, 1], qkv[:, :, 2]
    sc = np.einsum("bqhd,bkhd->bhqk", q, k) / np.sqrt(np.float32(DH))
    sc = np.where(mask, sc, np.float32(-1e9))
    sc = sc - sc.max(-1, keepdims=True)
    a = np.exp(sc)
    a = a / a.sum(-1, keepdims=True)
    ctx = np.einsum("bhqk,bkhd->bqhd", a, v).reshape(B, S, D)
    return (ctx.reshape(B * S, D) @ out_w + out_b).reshape(B, S, D).astype(np.float32)


def kernel(x, mask, qkv_w, qkv_b, out_w, out_b):
    global last_results
    x = np.asarray(x, dtype=np.float32)
    mask = np.asarray(mask).astype(bool)
    qkv_w = np.asarray(qkv_w, dtype=np.float32)
    qkv_b = np.asarray(qkv_b, dtype=np.float32)
    out_w = np.asarray(out_w, dtype=np.float32)
    out_b = np.asarray(out_b, dtype=np.float32)

    m2 = mask.reshape(S, S)
    if m2.all():
        causal = False
    elif np.array_equal(m2, np.tril(np.ones((S, S), dtype=bool))):
        causal = True
    else:
        return _numpy_fallback(x, mask, qkv_w, qkv_b, out_w, out_b)

    has_b = bool(np.any(qkv_b))
    dd = D + 1 if has_b else D
    nd = (dd + 127) // 128
    nc = _get_prog(causal, nd)

    in_maps = []
    for c in range(NCORES):
        b, hg = divmod(c, 4)
        hs = hg * HPC
        cols = slice(hs * DH, (hs + HPC) * DH)
        wc = np.concatenate(
            [qkv_w[:, cols], qkv_w[:, D:][:, cols], qkv_w[:, 2 * D :][:, cols]], axis=1
        )
        xtc = x[b].T
        if has_b:
            bc = np.concatenate(
                [qkv_b[cols], qkv_b[D:][cols], qkv_b[2 * D :][cols]]
            )
            wc = np.concatenate([wc, bc[None, :]], axis=0)
            xtc = np.concatenate([xtc, np.ones((1, S), np.float32)], axis=0)
        # zero-pad contraction dim to nd*128 and retile to [128, ...]
        pad = nd * 128 - xtc.shape[0]
        if pad:
            xtc = np.concatenate([xtc, np.zeros((pad, S), np.float32)], axis=0)
            wc = np.concatenate([wc, np.zeros((pad, E), np.float32)], axis=0)
        # xt: [dsub*128+p, ch*512+c] -> [p, ch, dsub, c]
        xt4 = np.ascontiguousarray(
            xtc.reshape(nd, 128, 4, 512).transpose(1, 2, 0, 3)
        ).astype(NP_BF16)
        w3 = np.ascontiguousarray(
            wc.reshape(nd, 128, E).transpose(1, 0, 2)
        ).astype(NP_BF16)
        wo3 = np.ascontiguousarray(
            out_w[cols, :].reshape(2, 128, D).transpose(1, 0, 2)
        ).astype(NP_BF16)
        in_maps.append({"xt": xt4, "w": w3, "wo": wo3})

    trace = os.environ.get("KERNEL_TRACE", "0") == "1"
    last_results = run_bass_kernel_spmd(
        nc, in_maps, core_ids=list(range(NCORES)), trace=trace
    )
    out = np.zeros((B, S, D), dtype=np.float32)
    for c in range(NCORES):
        out[c // 4] += last_results.results[c]["out"].astype(np.float32)
    out += out_b[None, None, :]
    return out
